# revision 17
# baseline (speedup 1.0000x reference)
"""Trainium2 Bass kernel for nn_DifferentiableSuperpixelTokenizer.

Self-contained: hardcodes all shapes (B=2, 3x224x224 images, 196 segments,
E=768). Sharding: 8 cores = (batch b in {0,1}) x (row-quarter q in {0..3});
each core computes conv features for its 56 image rows, partial segment
tables for all 196 segments, AllReduce(add/max) within the 4-core batch
group, then the full [196,768] fused output for its batch (host keeps one
copy per batch).

Per core:
  conv1 (3->64, 3x3, pad 1) + BN + ReLU   -> x       (f32, one-hot matmul grid)
  conv2 (64->768, 3x3, dil 2) + BN + ReLU -> feats   (fp16)
  edge conv on gray -> gradient_map (output) + similarity
  segment sums (exact, one-hot matmuls on PE), segment max via
  transpose-mode dma_gather of fp16 feature rows in segment-sorted slot
  order + strided vector tensor_reduce(max).
"""

import sys

sys.path.insert(0, "/opt/trn_rl_repo")

import numpy as np

import concourse.bacc as bacc
import concourse.tile as tile
import concourse.mybir as mybir
from concourse.bass_utils import run_bass_kernel_spmd

F32 = mybir.dt.float32
F32R = mybir.dt.float32r
F16 = mybir.dt.float16
I16 = mybir.dt.int16
AF = mybir.ActivationFunctionType
OP = mybir.AluOpType
AX = mybir.AxisListType

B, H, W = 2, 224, 224
M, E = 196, 768
BN_EPS = 1e-5

GW = 228               # padded grid width (real cols at 2..225)
QH = 56                # rows per core
HH = 28                # rows per half
HGRID = HH * GW        # 6384 real grid cells per half
HCELL = 6400           # padded half cells (50 subtiles of 128)
NPT = 50               # 128-px subtiles per half
ZROW = HCELL           # zero row index in per-half feats dram
FDR = HCELL + 128      # per-half feats dram rows (6528)

XROWS = 60
XL = XROWS * GW        # 13680
XG0 = 4                # xx2 head guard
XX2W = XG0 + XL + 940

IROWS = 62
IGL = IROWS * GW
IG0 = 8
IMGW = IG0 + IGL + 8

C1T = 456                                        # conv1/edge tile (2 rows)
C2T = 384                                        # conv2 pixel tile
C2TILES = [(i * C2T, C2T) for i in range(16)] + [(16 * C2T, 240)]

TAPS1 = [(dy, dx) for dy in (-1, 0, 1) for dx in (-1, 0, 1)]
DY2 = (-2, 0, 2)

_BUILD_CACHE = {}


# --------------------------------------------------------------------------
# host-side preparation
# --------------------------------------------------------------------------

def _host_prep(inputs):
    img = np.asarray(inputs["img"], np.float32)
    segments = np.asarray(inputs["segments"])
    centroids = np.asarray(inputs["centroids"], np.float32)

    inv1 = np.asarray(inputs["bn1_g"] / np.sqrt(inputs["bn1_v"] + BN_EPS),
                      np.float32)
    sh1 = np.asarray(inputs["conv1_b"] * inv1 + inputs["bn1_b"]
                     - inputs["bn1_m"] * inv1, np.float32)
    inv2 = np.asarray(inputs["bn2_g"] / np.sqrt(inputs["bn2_v"] + BN_EPS),
                      np.float32)
    sh2 = np.asarray(inputs["conv2_b"] * inv2 + inputs["bn2_b"]
                     - inputs["bn2_m"] * inv2, np.float32)

    bnc1 = np.stack([inv1, sh1], axis=1)                      # [64,2]
    bnc2 = np.zeros((128, 12), np.float32)
    for cc in range(6):
        bnc2[:, 2 * cc] = inv2[cc * 128:(cc + 1) * 128]
        bnc2[:, 2 * cc + 1] = sh2[cc * 128:(cc + 1) * 128]

    w1 = np.asarray(inputs["conv1_w"], np.float32)            # [64,3,3,3]
    we = np.asarray(inputs["edge_w"], np.float32)             # [2,1,3,3]
    w1e = np.zeros((27, 96), np.float16)
    for j, (dy, dx) in enumerate(TAPS1):
        for c in range(3):
            w1e[3 * j + c, 0:64] = w1[:, c, dy + 1, dx + 1]
            w1e[3 * j + c, 64:80] = we[0, 0, dy + 1, dx + 1] / 3.0
            w1e[3 * j + c, 80:96] = we[1, 0, dy + 1, dx + 1] / 3.0

    w2 = np.asarray(inputs["conv2_w"], np.float32)            # [768,64,3,3]
    w2pk = np.zeros((128, 36 * 128), np.float16)
    for cc in range(6):
        for mm in range(6):
            ky = mm % 3
            t = np.zeros((128, 128), np.float32)
            if mm < 3:
                t[0:64] = w2[cc * 128:(cc + 1) * 128, :, ky, 0].T
                t[64:128] = w2[cc * 128:(cc + 1) * 128, :, ky, 1].T
            else:
                t[0:64] = w2[cc * 128:(cc + 1) * 128, :, ky, 2].T
            w2pk[:, (cc * 6 + mm) * 128:(cc * 6 + mm + 1) * 128] = t

    fw = np.asarray(inputs["fusion_w"], np.float32)           # [768, 1536]
    fwt = np.zeros((128, 24 * 384), np.float16)
    for part in range(2):
        for cc in range(6):
            for dh in range(2):
                blk = fw[dh * 384:(dh + 1) * 384,
                         part * 768 + cc * 128:part * 768 + (cc + 1) * 128]
                k = (part * 6 + cc) * 2 + dh
                fwt[:, k * 384:(k + 1) * 384] = blk.T

    iota_rep = np.broadcast_to(np.arange(M, dtype=np.float16), (128, M)).copy()
    ident = np.eye(128, dtype=np.float16)
    onesg = np.zeros((32, 16), np.float16)
    for i in range(32):
        onesg[i, i % 16] = 1.0

    counts = np.stack([np.bincount(np.asarray(segments[b]).ravel(),
                                   minlength=M) for b in range(B)])
    inv_nK = (1.0 / np.maximum(counts, 1.0)).astype(np.float32)

    cn = np.stack([centroids[:, :, 0] / float(W),
                   centroids[:, :, 1] / float(H)], axis=-1)
    pos = (cn @ np.asarray(inputs["pos_w"], np.float32).T
           + np.asarray(inputs["pos_b"], np.float32))         # [2,196,768]

    fbrep = np.broadcast_to(np.asarray(inputs["fusion_b"], np.float32),
                            (128, 768)).copy()

    grids = []
    for b in range(B):
        g = np.zeros((3, H + 6, GW), np.float32)
        g[:, 3:3 + H, 2:2 + W] = img[b]
        grids.append(g)

    cell = (np.arange(HH)[:, None] * GW + np.arange(W)[None, :] + 2)

    # global slot size S
    S = 1
    segs_ch = {}
    for b in range(B):
        for q in range(4):
            for h in range(2):
                r0 = QH * q + HH * h
                seg = np.asarray(segments[b, r0:r0 + HH, :], np.int64)
                segs_ch[(b, q, h)] = seg
                S = max(S, int(np.bincount(seg.ravel(), minlength=M).max()))
    S = (S + 3) // 4 * 4
    LH = (M * S + 127) // 128 * 128
    IW = LH // 16

    in_maps = []
    for b in range(B):
        for q in range(4):
            r0c = QH * q
            imgp = np.zeros((3, IMGW), np.float16)
            imgp[:, IG0:IG0 + IGL] = grids[b][:, r0c:r0c + IROWS, :].reshape(
                3, IGL)

            xmask = np.ones((128, XROWS), np.float32)
            for r in range(XROWS):
                if not (0 <= r0c - 2 + r < H):
                    xmask[:, r] = 0.0

            segT = np.full((128, 100), -1.0, np.float16)
            idx16 = np.zeros((128, 2 * IW), np.int16)
            for h in range(2):
                seg = segs_ch[(b, q, h)]
                gm = np.full((HCELL,), -1, np.int64)
                gm[cell.ravel()] = seg.ravel()
                segT[:, 50 * h:50 * (h + 1)] = (
                    gm.reshape(50, 128).T.astype(np.float16))
                order = np.argsort(seg.ravel(), kind="stable")
                svals = seg.ravel()[order]
                rows = cell.ravel()[order]
                cnt = np.bincount(svals, minlength=M)
                off = np.zeros(M + 1, np.int64)
                np.cumsum(cnt, out=off[1:])
                sl = np.full((M, S), ZROW, np.int64)
                for k in range(M):
                    n = cnt[k]
                    if n:
                        sl[k, :n] = rows[off[k]:off[k] + n]
                idxs = np.full((LH,), ZROW, np.int64)
                idxs[:M * S] = sl.ravel()
                wrp = idxs.reshape(IW, 16).T.astype(np.int16)
                idx16[:, h * IW:(h + 1) * IW] = np.tile(wrp, (8, 1))

            inv6 = np.ascontiguousarray(
                np.tile(np.broadcast_to(inv_nK[b], (128, M)), (1, 6)),
                dtype=np.float32)
            posq2 = np.zeros((128, 1536), np.float32)
            posq2[:, 0:768] = pos[b, 0:128]
            posq2[0:68, 768:1536] = pos[b, 128:196]

            in_maps.append({
                "imgp": imgp, "w1e": w1e, "w2pk": w2pk, "fwt": fwt,
                "bnc1": bnc1, "bnc2": bnc2, "iota_rep": iota_rep,
                "ident": ident, "onesg": onesg, "segT": segT,
                "idx16": idx16, "inv6": inv6, "fbrep": fbrep,
                "posq2": posq2, "xmask": xmask,
            })
    return in_maps, {"S": S, "LH": LH, "IW": IW}


# --------------------------------------------------------------------------
# device module
# --------------------------------------------------------------------------

class _Ctx:
    pass


def _build(S, LH, IW):
    key = (S, LH, IW)
    if key in _BUILD_CACHE:
        return _BUILD_CACHE[key]

    nc = bacc.Bacc("TRN2", target_bir_lowering=False, debug=False,
                   num_devices=8)
    din = {}
    for name, shape, dt in [
        ("imgp", [3, IMGW], F16),
        ("w1e", [27, 96], F16),
        ("w2pk", [128, 36 * 128], F16),
        ("fwt", [128, 24 * 384], F16),
        ("bnc1", [64, 2], F32),
        ("bnc2", [128, 12], F32),
        ("iota_rep", [128, M], F16),
        ("ident", [128, 128], F16),
        ("onesg", [32, 16], F16),
        ("segT", [128, 100], F16),
        ("idx16", [128, 2 * IW], I16),
        ("inv6", [128, 6 * M], F32),
        ("fbrep", [128, 768], F32),
        ("posq2", [128, 1536], F32),
        ("xmask", [128, XROWS], F32),
    ]:
        din[name] = nc.dram_tensor(name, shape, dt, kind="ExternalInput").ap()
    demb = nc.dram_tensor("emb", [M, E], F32, kind="ExternalOutput").ap()
    dgrad = nc.dram_tensor("grad", [QH, W], F32, kind="ExternalOutput").ap()

    with tile.TileContext(nc) as tc:
        _emit(nc, tc, din, demb, dgrad, S, LH, IW)
    nc.compile()
    _BUILD_CACHE[key] = nc
    return nc


def _emit(nc, tc, din, demb, dgrad, S, LH, IW):
    from contextlib import ExitStack
    with ExitStack() as ctx:
        c = _Ctx()
        c.S, c.LH, c.IW = S, LH, IW
        c.pc = ctx.enter_context(tc.tile_pool(name="consts", bufs=1))
        c.pd = ctx.enter_context(tc.tile_pool(name="dram", bufs=1,
                                              space="DRAM"))
        c.pw = ctx.enter_context(tc.tile_pool(name="work", bufs=1))
        c.pa = ctx.enter_context(tc.tile_pool(name="phA", bufs=1))
        c.pfc = ctx.enter_context(tc.tile_pool(name="fc", bufs=2))
        c.pftf = ctx.enter_context(tc.tile_pool(name="ftf", bufs=2))
        c.poh = ctx.enter_context(tc.tile_pool(name="oh", bufs=2))
        c.ppa = ctx.enter_context(tc.tile_pool(name="psA", bufs=2,
                                               space="PSUM"))
        c.pps = ctx.enter_context(tc.tile_pool(name="psS", bufs=1,
                                               space="PSUM"))
        c.ppt = ctx.enter_context(tc.tile_pool(name="psT", bufs=2,
                                               space="PSUM"))
        c.pfw = ctx.enter_context(tc.tile_pool(name="fw", bufs=2))

        def load(name, shape, dt):
            t = c.pc.tile(shape, dt, tag=name)
            nc.sync.dma_start(out=t[:], in_=din[name][:])
            return t

        c.w1e = load("w1e", [27, 96], F16)
        c.w2pk = load("w2pk", [128, 36 * 128], F16)
        c.bnc1 = load("bnc1", [64, 2], F32)
        c.bnc2 = load("bnc2", [128, 12], F32)
        c.iota = load("iota_rep", [128, M], F16)
        c.ident = load("ident", [128, 128], F16)
        c.onesg = load("onesg", [32, 16], F16)
        c.segT = load("segT", [128, 100], F16)
        c.idx16 = load("idx16", [128, 2 * IW], I16)
        c.inv6 = load("inv6", [128, 6 * M], F32)
        c.xmask = load("xmask", [128, XROWS], F32)

        c.xx2 = c.pw.tile([128, XX2W], F16, tag="xx2")
        c.sumtab = c.pw.tile([128, 6 * M], F32, tag="sumtab")
        c.simrow = c.pw.tile([1, M], F32, tag="simrow")
        c.stt = c.pw.tile([128, 100], F16, tag="stt")
        c.tabm = [c.pw.tile([128, 6 * M], F16, tag=f"tabm{h}", name=f"tabm{h}")
                  for h in (0, 1)]
        c.fd = [c.pd.tile([FDR, E], F16, tag=f"fd{h}", name=f"fd{h}") for h in (0, 1)]

        nc.vector.memset(c.stt[:], 0.0)
        nc.vector.memset(c.xx2[:], 0.0)
        zr = c.pw.tile([1, E], F16, tag="zrow")
        nc.vector.memset(zr[:], 0.0)
        for h in (0, 1):
            nc.sync.dma_start(out=c.fd[h][ZROW:ZROW + 1, :], in_=zr[:])

        for h in (0, 1):
            _emit_half(nc, c, din, dgrad, h)

        maxtab = c.pw.tile([128, 6 * M], F32, tag="maxtab")
        nc.vector.tensor_tensor(out=maxtab[:], in0=c.tabm[0][:],
                                in1=c.tabm[1][:], op=OP.max)

        # ---- collectives
        cadd_i = c.pd.tile([128, 7 * M], F32)
        cadd_o = c.pd.tile([128, 7 * M], F32)
        cmax_i = c.pd.tile([128, 6 * M], F32)
        cmax_o = c.pd.tile([128, 6 * M], F32)
        stadd = c.pa.tile([128, 7 * M], F32, tag="stadd")
        nc.vector.memset(stadd[:], 0.0)
        nc.vector.tensor_copy(out=stadd[:, 0:6 * M], in_=c.sumtab[:])
        nc.vector.tensor_copy(out=stadd[0:1, 6 * M:7 * M], in_=c.simrow[:])
        nc.sync.dma_start(out=cadd_i[:], in_=stadd[:])
        nc.sync.dma_start(out=cmax_i[:], in_=maxtab[:])
        groups = [[0, 1, 2, 3], [4, 5, 6, 7]]
        nc.gpsimd.collective_compute("AllReduce", OP.add,
                                     replica_groups=groups,
                                     ins=[cadd_i[:]], outs=[cadd_o[:]])
        nc.gpsimd.collective_compute("AllReduce", OP.max,
                                     replica_groups=groups,
                                     ins=[cmax_i[:]], outs=[cmax_o[:]])
        # AR results come back into the staging/local tiles (reuse slots)
        nc.sync.dma_start(out=stadd[:], in_=cadd_o[:])
        nc.sync.dma_start(out=maxtab[:], in_=cmax_o[:])

        # ---- post: mean, W_k  (fp16 tables for the fusion matmuls)
        meanh = c.pa.tile([128, 6 * M], F16, tag="meanh")
        nc.vector.tensor_tensor(out=meanh[:], in0=stadd[:, 0:6 * M],
                                in1=c.inv6[:], op=OP.mult)
        maxh = c.pa.tile([128, 6 * M], F16, tag="maxh")
        nc.vector.tensor_copy(out=maxh[:], in_=maxtab[:])
        wk1 = c.pa.tile([1, M], F32, tag="wk1")
        nc.vector.tensor_tensor(out=wk1[:], in0=stadd[0:1, 6 * M:7 * M],
                                in1=c.inv6[0:1, 0:M], op=OP.mult)
        ones11 = c.pa.tile([1, 1], F32, tag="ones11")
        nc.vector.memset(ones11[:], 1.0)
        wkcol = []
        for sc, sgn in ((0, 128), (1, 68)):
            pk = c.ppt.tile([128, 128], F32, tag="aux", name="pswk")
            nc.tensor.matmul(out=pk[0:sgn, 0:1],
                             lhsT=wk1[:, sc * 128:sc * 128 + sgn],
                             rhs=ones11[:], start=True, stop=True)
            t = c.pa.tile([128, 1], F32, tag=f"wkcol{sc}")
            nc.vector.tensor_copy(out=t[0:sgn, :], in_=pk[0:sgn, 0:1])
            wkcol.append(t)

        # ---- fusion + output (fw tiles streamed from DRAM JIT)
        fbrep = c.pc.tile([128, 768], F32, tag="fbrep")
        nc.sync.dma_start(out=fbrep[:], in_=din["fbrep"][:])
        posq2 = c.pc.tile([128, 1536], F32, tag="posq2")
        nc.sync.dma_start(out=posq2[:], in_=din["posq2"][:])

        for dh in range(2):
            psf = [c.ppa.tile([128, 384], F32, tag="psA", name=f"psf{sc}")
                   for sc in range(2)]
            nmm = 0
            for part, tb in ((0, meanh), (1, maxh)):
                for cc in range(6):
                    k = (part * 6 + cc) * 2 + dh
                    nmm += 1
                    fwk = c.pfw.tile([128, 384], F16, tag="fwk",
                                     name=f"fwk{dh}_{k}")
                    nc.sync.dma_start(out=fwk[:],
                                      in_=din["fwt"][:, k * 384:(k + 1) * 384])
                    for sc, sgn in ((0, 128), (1, 68)):
                        nc.tensor.matmul(
                            out=psf[sc][0:sgn, :],
                            lhsT=tb[:, cc * M + sc * 128:
                                    cc * M + sc * 128 + sgn],
                            rhs=fwk[:],
                            start=(nmm == 1), stop=(nmm == 12))
            for sc, sgn in ((0, 128), (1, 68)):
                t1 = c.pa.tile([128, 384], F32, tag="fuse", name="t1", bufs=3)
                nc.vector.tensor_tensor(
                    out=t1[0:sgn, :], in0=psf[sc][0:sgn, :],
                    in1=fbrep[0:sgn, dh * 384:(dh + 1) * 384], op=OP.add)
                t2 = c.pa.tile([128, 384], F32, tag="fuse", name="t2", bufs=3)
                nc.scalar.activation(out=t2[0:sgn, :], in_=t1[0:sgn, :],
                                     func=AF.Copy, scale=wkcol[sc][0:sgn, :])
                t3 = c.pa.tile([128, 384], F32, tag="fuse", name="t3", bufs=3)
                nc.vector.tensor_tensor(
                    out=t3[0:sgn, :], in0=t2[0:sgn, :],
                    in1=posq2[0:sgn, sc * 768 + dh * 384:
                              sc * 768 + (dh + 1) * 384], op=OP.add)
                nc.sync.dma_start(
                    out=demb[sc * 128:sc * 128 + sgn,
                             dh * 384:(dh + 1) * 384],
                    in_=t3[0:sgn, :])


def _emit_half(nc, c, din, dgrad, h):
    S, LH, IW = c.S, c.LH, c.IW
    if h == 0:
        xr0, nxr = 0, 32          # x rows covered by xx1
        c1r0, c1n = 0, 32         # x rows computed by conv1
    else:
        xr0, nxr = 29, 31
        c1r0, c1n = 32, 28

    xx1 = c.pw.tile([128, LH], F16, tag="bigAB", name=f"xx1_{h}",
                    bufs=2)
    for j, (dy, dx) in enumerate(TAPS1):
        off = IG0 + (xr0 + 1 + dy) * GW + dx
        nc.sync.dma_start(out=xx1[3 * j:3 * j + 3, 0:nxr * GW],
                          in_=din["imgp"][:, off:off + nxr * GW])

    # ---- conv1 -> xx2[0:64]
    n1t = c1n * GW // C1T
    for t in range(n1t):
        ps = c.ppa.tile([128, C1T], F32, tag="psA")
        ro = (c1r0 - xr0) * GW + t * C1T
        nc.tensor.matmul(out=ps[0:64, :], lhsT=c.w1e[:, 0:64],
                         rhs=xx1[0:27, ro:ro + C1T],
                         start=True, stop=True)
        o0 = XG0 + c1r0 * GW + t * C1T
        nc.scalar.activation(out=c.xx2[0:64, o0:o0 + C1T], in_=ps[0:64, :],
                             func=AF.Relu, scale=c.bnc1[:, 0:1],
                             bias=c.bnc1[:, 1:2])
    # boundary-row mask + pad cols
    xv = c.xx2[0:64, XG0 + c1r0 * GW:XG0 + (c1r0 + c1n) * GW].rearrange(
        "p (r w) -> p r w", w=GW)
    mk = c.xmask[0:64, c1r0:c1r0 + c1n].to_broadcast([64, c1n, GW])
    nc.vector.tensor_tensor(out=xv, in0=xv, in1=mk, op=OP.mult)
    nc.vector.memset(xv[:, :, 0:2], 0.0)
    nc.vector.memset(xv[:, :, 226:228], 0.0)
    # replica at addr-2: addr XG0+g holds x[g] on 0:64 and x[g+2] on 64:128
    nc.sync.dma_start(
        out=c.xx2[64:128, XG0 - 2 + c1r0 * GW:XG0 - 2 + (c1r0 + c1n) * GW],
        in_=c.xx2[0:64, XG0 + c1r0 * GW:XG0 + (c1r0 + c1n) * GW])

    # ---- edge conv -> gradient + similarity (per 2-row tile)
    eb = c.pw.tile([16, 2], F32, tag="ebias", name="eb")
    nc.vector.memset(eb[:, 0:1], 1e-8)
    nc.vector.memset(eb[:, 1:2], 1.0)
    sim16 = c.pw.tile([16, HGRID], F16, tag="sim16", name=f"sim16_{h}")
    for t in range(HGRID // C1T):
        ps = c.ppa.tile([128, C1T], F32, tag="psA")
        eoff = (2 * GW if h == 0 else GW) + t * C1T
        nc.tensor.matmul(out=ps[0:32, :], lhsT=c.w1e[:, 64:96],
                         rhs=xx1[0:27, eoff:eoff + C1T],
                         start=True, stop=True)
        sq = c.pfc.tile([32, C1T], F16, tag="sq32")
        nc.scalar.activation(out=sq[:], in_=ps[0:32, :], func=AF.Square)
        g2p = c.ppt.tile([128, C1T], F32, tag="aux", name="g2p")
        nc.tensor.matmul(out=g2p[0:16, :], lhsT=c.onesg[:],
                         rhs=sq[:], start=True, stop=True)
        grt = c.pfc.tile([16, C1T], F32, tag="grt")
        nc.scalar.activation(out=grt[:], in_=g2p[0:16, :], func=AF.Sqrt,
                             bias=eb[:, 0:1])
        smt = c.pfc.tile([16, C1T], F32, tag="smt")
        nc.scalar.activation(out=smt[:], in_=grt[:], func=AF.Relu,
                             scale=-1.0, bias=eb[:, 1:2])
        nc.vector.tensor_scalar_min(sim16[:, t * C1T:(t + 1) * C1T], smt[:],
                                    1.0)
        gv = grt[0:1, :].rearrange("p (r w) -> p r w", w=GW)
        nc.sync.dma_start(out=dgrad[HH * h + 2 * t:HH * h + 2 * t + 2, :],
                          in_=gv[:, :, 2:226])

    # ---- conv2 + transposes + sums
    psums = [c.pps.tile([128, 2 * M], F32, tag=f"ps{cp}", name=f"ps{cp}")
             for cp in range(3)]
    psim = c.pps.tile([1, M], F32, tag="psim")
    subi = 0
    for (p0, pw_) in C2TILES:
        fcc = []
        for cc in range(6):
            ps = c.ppa.tile([128, C2T], F32, tag="psA")
            for mm in range(6):
                ky = mm % 3
                dy = DY2[ky]
                dxb = -2 if mm < 3 else 2
                off = XG0 + (2 + dy) * GW + dxb + h * HGRID + p0
                nc.tensor.matmul(
                    out=ps[:, 0:pw_],
                    lhsT=c.w2pk[:, (cc * 6 + mm) * 128:
                                (cc * 6 + mm + 1) * 128],
                    rhs=c.xx2[:, off:off + pw_],
                    start=(mm == 0), stop=(mm == 5))
            f = c.pfc.tile([128, C2T], F16, tag=f"fc{cc}")
            nc.scalar.activation(out=f[:, 0:pw_], in_=ps[:, 0:pw_],
                                 func=AF.Relu,
                                 scale=c.bnc2[:, 2 * cc:2 * cc + 1],
                                 bias=c.bnc2[:, 2 * cc + 1:2 * cc + 2])
            fcc.append(f)
        nsub = (pw_ + 127) // 128
        for sub in range(nsub):
            sw = min(128, pw_ - sub * 128)
            gcol = 50 * h + subi
            ftf = c.pftf.tile([128, E], F16, tag="ftf")
            if sw < 128:
                nc.vector.memset(ftf[:], 0.0)
            for cc in range(6):
                pt_ = c.ppt.tile([128, 128], F16, tag="aux", name="ptT")
                nc.tensor.transpose(
                    out=pt_[0:sw, 0:128],
                    in_=fcc[cc][:, sub * 128:sub * 128 + sw],
                    identity=c.ident[:])
                nc.vector.tensor_copy(out=ftf[0:sw, cc * 128:(cc + 1) * 128],
                                      in_=pt_[0:sw, 0:128])
            pt2 = c.ppt.tile([128, 128], F16, tag="aux", name="pt2")
            nc.tensor.transpose(out=pt2[0:sw, 0:16],
                                in_=sim16[:, subi * 128:subi * 128 + sw],
                                identity=c.ident[0:16, 0:16])
            nc.vector.tensor_copy(out=c.stt[0:sw, gcol:gcol + 1],
                                  in_=pt2[0:sw, 0:1])
            nc.sync.dma_start(
                out=c.fd[h][subi * 128:subi * 128 + sw, :], in_=ftf[0:sw, :])
            oh = c.poh.tile([128, M], F16, tag="oh")
            nc.vector.tensor_tensor(
                out=oh[:],
                in0=c.segT[:, gcol:gcol + 1].to_broadcast([128, M]),
                in1=c.iota[:], op=OP.is_equal)
            last = (subi == NPT - 1)
            for cc in range(6):
                cp, side = divmod(cc, 2)
                # two accumulation chains share one psum bank: only the
                # side-0 chain's first matmul marks the bank pending-zero;
                # side-1's first write then overwrites its own (pending)
                # bytes, both chains accumulate afterwards.
                nc.tensor.matmul(
                    out=psums[cp][:, side * M:(side + 1) * M],
                    lhsT=ftf[:, cc * 128:(cc + 1) * 128], rhs=oh[:],
                    start=(subi == 0 and side == 0), stop=last,
                    skip_group_check=True)
            nc.tensor.matmul(out=psim[:], lhsT=c.stt[:, gcol:gcol + 1],
                             rhs=oh[:], start=(subi == 0), stop=last)
            subi += 1

    for cp in range(3):
        if h == 0:
            nc.vector.tensor_copy(out=c.sumtab[:, cp * 2 * M:(cp + 1) * 2 * M],
                                  in_=psums[cp][:])
        else:
            nc.vector.tensor_tensor(
                out=c.sumtab[:, cp * 2 * M:(cp + 1) * 2 * M],
                in0=c.sumtab[:, cp * 2 * M:(cp + 1) * 2 * M],
                in1=psums[cp][:], op=OP.add)
    if h == 0:
        nc.vector.tensor_copy(out=c.simrow[:], in_=psim[:])
    else:
        nc.vector.tensor_tensor(out=c.simrow[:], in0=c.simrow[:],
                                in1=psim[:], op=OP.add)

    # ---- gather + max reduce
    for cc in range(6):
        g = c.pw.tile([128, LH], F16, tag="bigAB", name=f"gath{h}_{cc}",
                      bufs=2)
        nc.gpsimd.dma_gather(
            out_ap=g[:].rearrange("p (a l) -> p a l", a=1),
            in_ap=c.fd[h][:, cc * 128:(cc + 1) * 128],
            idxs_ap=c.idx16[:, h * IW:(h + 1) * IW],
            num_idxs=LH, num_idxs_reg=LH,
            elem_size=128, elem_step=E, transpose=True,
            single_packet=False)
        nc.vector.tensor_reduce(
            out=c.tabm[h][:, cc * M:(cc + 1) * M],
            in_=g[:, 0:M * S].rearrange("p (k s) -> p k s", s=S),
            axis=AX.X, op=OP.max)


# --------------------------------------------------------------------------
# entry point
# --------------------------------------------------------------------------

def kernel(**inputs):
    in_maps, meta = _host_prep(inputs)
    nc = _build(meta["S"], meta["LH"], meta["IW"])
    res = run_bass_kernel_spmd(nc, in_maps, core_ids=list(range(8)))
    segments = np.asarray(inputs["segments"])
    final = np.zeros((B, M, E), np.float32)
    grad = np.zeros((B, H, W), np.float32)
    for b in range(B):
        final[b] = res.results[b * 4]["emb"]
        for q in range(4):
            grad[b, QH * q:QH * (q + 1), :] = res.results[b * 4 + q]["grad"]
    return final, grad, segments
